# revision 44
# baseline (speedup 1.0000x reference)
"""DownscaleLabel Trainium2 kernel (v3: raw HWDGE load + fp8 DoubleRow pool).

Input:  label [8, 1024, 1024] int32, values in [-1, 6] (-1 = ignore).
Output: [8, 1, 64, 64] int32. Per 16x16 block: the dominant real class c
        (0..6) if its pixel count >= 192 (= 0.75 * 256), else -1.

Since 192 > 128, at most one class can reach the threshold, so
    out = -1 + sum_c (c+1) * [count_c >= 192]
needs no argmax or tie-breaking.

Per-core plan (one 1024x1024 image per NeuronCore, batch-sharded):

 - Input DMA: raw int32 on the SP HWDGE ring (no gpsimd SWDGE descriptor
   generation), in column chunks; each image row contributes one >=512B
   descriptor so the 16 DMA engines run at full rate and transfers start
   ~1.5us into the kernel.
 - Row-paired layout: image row r lands in partition (r % 256)//2, free
   slot (r//256)*2048 + (r%2)*1024 + col.  A single fp8 DoubleRow matmul
   then contracts (partition, j) = 256 rows, i.e. all 16 rows of a block
   row, per super-tile t4 = r//256.
 - Encodes produce fp8-e5m2 bit patterns 2^(15-5k) as uint8:
     el8 = -20*x + 120  on ACT (class 0..3 -> exp 15,10,5,0; x>=4 junk
           fractions; x=-1 -> byte 140 = -2^-12, absorbed by a +1/16 vi
           bias)
     eh8 = max(20*x, 0) on DVE (class 3..6 -> exp 0,5,10,15; x<=2 junk)
 - PE fp8e5 DoubleRow matmuls vs block-diagonal 1.0 stationaries row-pool
   16 rows into PSUM fp32 (fields <= 16, exact); lo plane partitions
   0:64, hi 64:128, one PSUM bank per chunk.
 - vi (ACT) converts psum + 0.0625 -> int32; fkw (gpsimd) extracts field
   pairs 10 bits apart; reds (gpsimd) col-pools 16 -> rw [128, 2*64].
 - Threshold chain (DVE, 6 ops) as in v2 with remapped magic weights;
   PE fold matmul adds partitions p and p+64; ACT subtracts 1; int32
   [64, 64] result DMAs out on the ACT ring.
"""

import sys

import numpy as np

_BASS_REPO = "/opt/trn_rl_repo"

H = W = 1024
SC = 16
TH = TW = 64
P = 128
N_CORES = 8

# DMA chunks: 4 column quarters x 4 super-tiles = 16 transfers on the SP
# ring (issue cost ~620ns each stays ahead of the ~730ns/dma transfer rate)
DMA_COLS = [(0, 256), (256, 256), (512, 256), (768, 256)]
# compute chunks (c0, cw); cw multiple of 16, 2*cw <= 512 (one psum bank),
# nested inside DMA quarters; small tail chunks shorten the drain chain
CHUNKS = [(0, 256), (256, 256), (512, 256), (768, 128), (896, 64), (960, 64)]
NCH = len(CHUNKS)
# engine for each chunk's eh8 encode: ACT / DVE balance (gpsimd has no
# usable ALU ops on this toolchain); chunk 0 on DVE so PE starts earliest
EH_ENG = ["dve", "act", "act", "act", "dve", "dve"]
# psum col offset per chunk: both planes fit the chunk's own 2KB bank
PBANK = [512 * i for i in range(NCH)]
PSCR = 512 * NCH      # scratch bank for PE p-state warmup matmuls
N_WARM = 12           # dummy DR matmuls bridging the DMA fill phase
W_WARM = 192

PAIRMASK = 31 | (31 << 10)   # 0x7C1F
FLAG_C = 320 * 1025          # +320 per 10-bit field: bit9/bit19 = count>=192
M_MASK = 0x401               # flag word: bits 0,10 after >>9
# magic multipliers (see smalls): weight = class id + 1 for the set flag bit
M_LO = (3 << 13) | (4 << 10) | (1 << 3) | 2   # bits {0,3,10,13} -> c2,c3,c0,c1
M_HI = (5 << 13) | (0 << 10) | (7 << 3) | 6   # bits {0,3,10,13} -> c4,dup,c6,c5


def _ensure_path():
    if _BASS_REPO not in sys.path:
        sys.path.insert(0, _BASS_REPO)


def make_consts():
    """Host-side constant tensors fed as kernel inputs."""
    # Four DoubleRow stationaries (one per super-tile t4), each [128, 2, 64]
    # in fp8e5 (uint8 bytes, 1.0 = 0x3C): partition p routes to out partition
    # m = 16*t4 + p//8 = block-row, for both j.  All matmuls target psum
    # partitions 0:64 (walrus only supports DR at psum base 0) and
    # accumulate; planes are separated by psum COLUMNS, not partitions.
    pw8 = np.zeros((P, 4 * 2 * TW), dtype=np.uint8)
    k = np.arange(P)
    for t4 in range(4):
        for j in range(2):
            pw8[k, t4 * 128 + j * 64 + 16 * t4 + k // 8] = 0x3C
    return (pw8,)


def emit_downscale(ctx, tc, out_ap, label_ap, pw8_ap):
    """Emit the per-core kernel body into TileContext tc."""
    _ensure_path()
    from concourse import mybir
    from concourse.alu_op_type import AluOpType as aop

    nc = tc.nc
    dt = mybir.dt

    cpool = ctx.enter_context(tc.tile_pool(name="consts", bufs=1))
    xpool = ctx.enter_context(tc.tile_pool(name="x", bufs=1))
    epool = ctx.enter_context(tc.tile_pool(name="e", bufs=1))
    ppool = ctx.enter_context(tc.tile_pool(name="psum", bufs=1, space="PSUM"))
    spool = ctx.enter_context(tc.tile_pool(name="small", bufs=1))

    # ---- consts (SP HWDGE ring, issued first so warmups start early) ----
    pw8 = cpool.tile([P, 4 * 2 * TW], dt.uint8)
    nc.sync.dma_start(pw8[:], pw8_ap)

    # ---- tiles ----
    x = xpool.tile([P, 8 * W], dt.int32, tag="x")
    el8 = epool.tile([P, 8 * W], dt.uint8, tag="el8")
    eh8 = epool.tile([P, 8 * W], dt.uint8, tag="eh8")
    scr8 = epool.tile([P, 2 * W_WARM], dt.uint8, tag="scr8")
    psum = ppool.tile([P, PSCR + 512], dt.float32)
    # downstream tiles: partition = block-row g; planes column-separated
    vi_t = spool.tile([TH, 2 * W], dt.int32, tag="vi")
    fkw_t = spool.tile([TH, 4 * W], dt.int32, tag="fkw")
    rw = spool.tile([TH, 4 * TW], dt.int32, tag="rw")
    resi = spool.tile([TH, TW], dt.int32, tag="resi")
    st = spool.tile([TH, 4 * TW], dt.int32, tag="st")
    sm = spool.tile([TH, 4 * TW], dt.int32, tag="sm")
    sv = spool.tile([TH, 2 * TW], dt.int32, tag="sv")
    svm = spool.tile([TH, 2 * TW], dt.int32, tag="svm")
    ssh = spool.tile([TH, 2 * TW], dt.int32, tag="ssh")

    # row-major views: r = 2*t4 + j indexes the 8 rows per partition
    x_r = x[:, :].rearrange("p (r c) -> p r c", r=8)
    el_r = el8[:, :].rearrange("p (r c) -> p r c", r=8)
    eh_r = eh8[:, :].rearrange("p (r c) -> p r c", r=8)

    # ---- input DMA: SP HWDGE ring, raw int32, row-paired layout ----
    # (image row 256*t4 + 2*p + j -> partition p, free slot (2*t4+j)*1024 + c;
    #  one transfer per (chunk, super-tile) keeps the DMA APs at 3 dims)
    x_4 = x[:, :].rearrange("p (t j c) -> p t j c", t=4, j=2)
    for (c0, cw) in DMA_COLS:
        for t4 in range(4):
            nc.sync.dma_start(
                x_4[:, t4, :, c0 : c0 + cw],
                label_ap[256 * t4 : 256 * (t4 + 1), c0 : c0 + cw].rearrange(
                    "(p j) c -> p j c", j=2
                ),
            )

    def encode_el(c0, cw):
        nc.scalar.activation(
            el_r[:, :, c0 : c0 + cw],
            x_r[:, :, c0 : c0 + cw],
            mybir.ActivationFunctionType.Copy,
            bias=120.0,
            scale=-20.0,
        )

    def encode_eh(ci):
        c0, cw = CHUNKS[ci]
        if EH_ENG[ci] == "act":
            nc.scalar.activation(
                eh_r[:, :, c0 : c0 + cw],
                x_r[:, :, c0 : c0 + cw],
                mybir.ActivationFunctionType.Relu,
                bias=0.0,
                scale=20.0,
            )
        else:
            nc.vector.tensor_scalar(
                eh_r[:, :, c0 : c0 + cw],
                x_r[:, :, c0 : c0 + cw],
                20,
                0,
                aop.mult,
                aop.max,
            )

    def warmups():
        # PE p-state warmup: harmless DR matmuls on zeroed scratch keep the
        # array continuously busy through the DMA fill phase so the clock
        # ramps before real work arrives.
        nc.vector.memset(scr8[:], 0)
        stat = (
            pw8[:, 0:128].rearrange("p (j m) -> p j m", j=2).bitcast(dt.float8e5)
        )
        mov = scr8[:, :].rearrange("p (j c) -> p j c", j=2).bitcast(dt.float8e5)
        for _ in range(N_WARM):
            nc.tensor.matmul(
                psum[0:TH, PSCR : PSCR + W_WARM],
                stat,
                mov,
                start=True,
                stop=True,
                perf_mode=mybir.MatmulPerfMode.DoubleRow,
                skip_group_check=True,
            )

    def mms(ci):
        # 8-matmul accumulation group into psum[0:64, bank ci]: el plane at
        # cols [0, cw), eh at [cw, 2cw); out partition = block-row g.
        c0, cw = CHUNKS[ci]
        for t4 in range(4):
            stat = (
                pw8[:, t4 * 128 : (t4 + 1) * 128]
                .rearrange("p (j m) -> p j m", j=2)
                .bitcast(dt.float8e5)
            )
            for plane, e8 in ((0, el8), (1, eh8)):
                mov = (
                    e8[:, t4 * 2048 : (t4 + 1) * 2048]
                    .rearrange("p (j c) -> p j c", j=2)[:, :, c0 : c0 + cw]
                    .bitcast(dt.float8e5)
                )
                nc.tensor.matmul(
                    psum[
                        0:TH,
                        PBANK[ci] + plane * cw : PBANK[ci] + (plane + 1) * cw,
                    ],
                    stat,
                    mov,
                    start=(t4 == 0 and plane == 0),
                    stop=(t4 == 3 and plane == 1),
                    perf_mode=mybir.MatmulPerfMode.DoubleRow,
                    skip_group_check=True,
                )

    def vi_op(ci):
        c0, cw = CHUNKS[ci]
        nc.scalar.activation(
            vi_t[:, 2 * c0 : 2 * (c0 + cw)],
            psum[0:TH, PBANK[ci] : PBANK[ci] + 2 * cw],
            mybir.ActivationFunctionType.Copy,
            bias=0.0625,
            scale=1.0,
        )

    def fkws(ci):
        c0, cw = CHUNKS[ci]
        nc.vector.tensor_scalar(
            fkw_t[:, 2 * c0 : 2 * (c0 + cw)],
            vi_t[:, 2 * c0 : 2 * (c0 + cw)],
            5,
            PAIRMASK,
            aop.logical_shift_right,
            aop.bitwise_and,
        )
        nc.vector.tensor_scalar(
            fkw_t[:, 2 * W + 2 * c0 : 2 * W + 2 * (c0 + cw)],
            vi_t[:, 2 * c0 : 2 * (c0 + cw)],
            PAIRMASK,
            None,
            aop.bitwise_and,
        )

    def reds(ci):
        # col-pool 16: fkw [64, (plane, b, 16)] -> rw [64, jj*128 + pl*64 + b]
        c0, cw = CHUNKS[ci]
        for jj in range(2):
            with nc.allow_low_precision(reason="small int counts, exact"):
                nc.vector.tensor_reduce(
                    rw[:, 128 * jj : 128 * jj + 128].rearrange(
                        "g (pl b) -> g pl b", pl=2
                    )[:, :, c0 // SC : (c0 + cw) // SC],
                    fkw_t[
                        :, 2 * W * jj + 2 * c0 : 2 * W * jj + 2 * (c0 + cw)
                    ].rearrange("g (pl b s) -> g pl b s", pl=2, s=SC),
                    mybir.AxisListType.X,
                    aop.add,
                )

    def smalls(b0, b1):
        # threshold pass (DVE) over block-columns [b0, b1); planes are
        # column-blocks.  u indexes (pair j, plane).
        v = nc.vector
        bw = b1 - b0

        def w4(t):
            return t[:, :].rearrange("g (u b) -> g u b", u=4)[:, :, b0:b1]

        def w2(t):
            return t[:, :].rearrange("g (u b) -> g u b", u=2)[:, :, b0:b1]

        v.tensor_scalar(w4(st), w4(rw), FLAG_C, None, aop.add)
        v.tensor_scalar(
            w4(sm), w4(st), 9, M_MASK, aop.logical_shift_right, aop.bitwise_and
        )
        # V = m_pair0 + 8 * m_pair1 -> one-hot bits {0,3,10,13}
        v.scalar_tensor_tensor(
            w2(sv),
            sm[:, 128:256].rearrange("g (u b) -> g u b", u=2)[:, :, b0:b1],
            8,
            sm[:, 0:128].rearrange("g (u b) -> g u b", u=2)[:, :, b0:b1],
            aop.mult,
            aop.add,
        )
        v.tensor_scalar(svm[:, b0:b1], sv[:, b0:b1], M_LO, None, aop.mult)
        v.tensor_scalar(
            svm[:, TW + b0 : TW + b1], sv[:, TW + b0 : TW + b1], M_HI, None, aop.mult
        )
        v.tensor_scalar(
            w2(ssh), w2(svm), 13, 7, aop.logical_shift_right, aop.bitwise_and
        )
        # out = w_el + w_eh - 1  (at most one weight nonzero)
        v.scalar_tensor_tensor(
            resi[:, b0:b1],
            ssh[:, b0:b1],
            -1,
            ssh[:, TW + b0 : TW + b1],
            aop.add,
            aop.add,
        )
        nc.sync.dma_start(out_ap[:, b0:b1], resi[:, b0:b1])

    # ---- pipeline emission ----
    warmups()
    for ci, (c0, cw) in enumerate(CHUNKS):
        encode_el(c0, cw)
        if ci >= 1:
            vi_op(ci - 1)
            fkws(ci - 1)
            reds(ci - 1)
            if ci - 1 == 2:
                # wave A: block-cols of chunks 0-2 finish early
                smalls(0, 48)
        encode_eh(ci)
        mms(ci)
    vi_op(NCH - 1)
    fkws(NCH - 1)
    reds(NCH - 1)
    smalls(48, TW)


def _split_multi_waits(nc):
    """This toolchain's walrus codegen accepts at most ONE semaphore wait per
    engine instruction (two on EventSemaphore).  The Tile scheduler sometimes
    emits more; spill the extras onto same-engine NoOp carriers inserted just
    before the instruction (engines dispatch in order, so the carrier's wait
    is satisfied before the instruction issues -- semantics preserved)."""
    _ensure_path()
    from concourse import mybir

    for func in nc.m.functions:
        for blk in func.blocks:
            insts = blk.instructions
            out = []
            changed = False
            for ins in insts:
                si = ins.sync_info
                cap = 2 if isinstance(ins, mybir.InstEventSemaphore) else 1
                if si and si.on_wait and len(si.on_wait) > cap:
                    waits = list(si.on_wait)
                    for w in waits[:-cap]:
                        out.append(
                            mybir.InstNoOp(
                                name=nc.get_next_instruction_name(),
                                engine=ins.engine,
                                sync_info=mybir.SyncInfo(on_wait=[w], on_update=[]),
                                bass_nofuse=True,
                            )
                        )
                    si.on_wait = waits[-cap:]
                    changed = True
                out.append(ins)
            if changed:
                blk.instructions = out


def _install_ntff_hook():
    """Provide antenv.axon_hooks + the ctypes NTFF profile hook when the
    agent image lacks them (mirrors trn_agent_boot.trn_boot section 6)."""
    import contextlib
    import ctypes
    import types

    try:
        from antenv.axon_hooks import get_axon_ntff_profile_hook  # noqa: F401

        return
    except ImportError:
        pass
    _ensure_path()
    import antenv

    so_path = "/opt/axon/libaxon_pjrt.so"
    try:
        lib = ctypes.CDLL(so_path)
    except OSError:
        return
    if not hasattr(lib, "axon_start_nrt_profile"):
        return
    lib.axon_start_nrt_profile.argtypes = [
        ctypes.POINTER(ctypes.c_int64),
        ctypes.c_size_t,
    ]
    lib.axon_start_nrt_profile.restype = ctypes.c_int64
    lib.axon_stop_nrt_profile.argtypes = [ctypes.c_char_p]
    lib.axon_stop_nrt_profile.restype = ctypes.c_int64

    @contextlib.contextmanager
    def _hook(output_dir, device_ids):
        import jax

        jax.devices()
        if device_ids:
            ids = (ctypes.c_int64 * len(device_ids))(*device_ids)
            rc = lib.axon_start_nrt_profile(ids, len(device_ids))
        else:
            rc = lib.axon_start_nrt_profile(None, 0)
        if rc != 0:
            raise RuntimeError(f"axon_start_nrt_profile rc={rc}")
        try:
            yield
        finally:
            n = lib.axon_stop_nrt_profile(str(output_dir).encode())
            print(f"ntff profile: {n} file(s) written to {output_dir}", file=sys.stderr)

    mod = types.ModuleType("antenv.axon_hooks")
    _h = [_hook]
    mod.set_axon_ntff_profile_hook = lambda h: _h.__setitem__(0, h)
    mod.get_axon_ntff_profile_hook = lambda: _h[0]
    sys.modules["antenv.axon_hooks"] = mod
    antenv.axon_hooks = mod

    # upload_artifacts pushes the NEFF dir to a cloud bucket; keep local.
    from concourse import bass_utils as _bu

    _bu.upload_artifacts = lambda tmpdir: tmpdir


_NC_CACHE = None


def _build_nc(split_waits=True):
    global _NC_CACHE
    if _NC_CACHE is not None:
        return _NC_CACHE
    _ensure_path()
    from contextlib import ExitStack

    import concourse.bass as bass
    import concourse.tile as tile
    from concourse import mybir

    dt = mybir.dt
    nc = bass.Bass("TRN2", target_bir_lowering=False, debug=False)
    label = nc.dram_tensor("label", [H, W], dt.int32, kind="ExternalInput").ap()
    pw8 = nc.dram_tensor("pw8", [P, 4 * 2 * TW], dt.uint8, kind="ExternalInput").ap()
    out = nc.dram_tensor("out", [TH, TW], dt.int32, kind="ExternalOutput").ap()
    with tile.TileContext(nc) as tc:
        with ExitStack() as ctx:
            emit_downscale(ctx, tc, out, label, pw8)
    if split_waits:
        _split_multi_waits(nc)
        _NC_CACHE = nc
    return nc


def run_on_hw(label, trace=False):
    """Run on the 8 NeuronCores; returns (out [8,1,64,64] int32, exec_time_ns)."""
    _ensure_path()
    from concourse.bass_utils import run_bass_kernel_spmd

    if trace:
        _install_ntff_hook()
    nc = _build_nc()
    (pw8,) = make_consts()
    label = np.ascontiguousarray(label, dtype=np.int32)
    in_maps = [{"label": label[i], "pw8": pw8} for i in range(N_CORES)]
    r = run_bass_kernel_spmd(nc, in_maps, core_ids=list(range(N_CORES)), trace=trace)
    outs = np.stack([r.results[i]["out"] for i in range(N_CORES)])
    return outs.reshape(8, 1, TH, TW).astype(np.int32), r.exec_time_ns


def kernel(label):
    out, _ = run_on_hw(label, trace=False)
    return out
